# revision 1
# baseline (speedup 1.0000x reference)
"""Trainium2 Bass kernel for nn_Encoder_88983132439258 (GNN message passing).

Strategy (8 NeuronCores, data-parallel over destination nodes = graphs):
  - Each core owns 8192 destination nodes (= 2 complete graphs).
  - Host pre-builds a padded slot table (dst-major CSR padded to PAD slots per
    dst) so the per-layer edge aggregation becomes:
        indirect-DMA gather of z[src] rows  ->  DVE w-scale  ->  DVE slot-reduce
  - Weights are folded into the gather table: the table for layer l holds
    z_l = h_l @ W_l, so a conv layer is  h_{l+1} = ELU(segsum(w * z_l[src]) + b_l).
  - Per-layer epilogue runs feature-major via PE transposes and a block-diagonal
    (4x32x32) weight matmul, then the new table is AllGathered across cores.
  - FC head: h4 is AllGathered as H [16, 131072]; each core computes a 32-wide
    column shard of FC1 and its FC2 partial; partials are AllReduced.
"""

import numpy as np
import ml_dtypes

import concourse.bacc as bacc
import concourse.mybir as mybir
import concourse.tile as tile
import concourse.bass as bass
from concourse import bass_utils

F32 = mybir.dt.float32
BF16 = mybir.dt.bfloat16
I32 = mybir.dt.int32

N = 65536
NODES_PER = 4096
N_GRAPHS = 16
E_TOT = 2097152
FEAT_IN = 16
HID = 32
FC_HID = 256
LATENT = 64
NC = 8                 # cores
OWN = N // NC          # 8192 dsts per core
NCHUNK = 16            # dst chunks per core
CHD = OWN // NCHUNK    # 512 dsts per chunk
NB = CHD // 128        # 4 dst blocks of 128 per chunk
P = 128

_prog_cache = {}


def _build_program(PAD):
    """One SPMD program for all 8 cores; per-core data comes via inputs."""
    PAD4 = PAD * NB
    nc = bacc.Bacc("TRN2", target_bir_lowering=False, debug=False,
                   num_devices=NC)

    # ---- I/O ----
    tab1 = nc.dram_tensor("tab1", [N + 1, HID], BF16, kind="ExternalInput")
    idx_in = nc.dram_tensor("idx", [NCHUNK, P, PAD4], I32,
                            kind="ExternalInput")
    w_in = nc.dram_tensor("warr", [NCHUNK, P, PAD4], BF16,
                          kind="ExternalInput")
    wbd_in = nc.dram_tensor("wbd", [2, P, P], BF16, kind="ExternalInput")
    bst_in = nc.dram_tensor("bst", [3, P], F32, kind="ExternalInput")
    idf_in = nc.dram_tensor("identf", [P, P], F32, kind="ExternalInput")
    idb_in = nc.dram_tensor("identb", [P, P], BF16, kind="ExternalInput")
    wfc1_in = nc.dram_tensor("wfc1s", [P, 1024 * HID], BF16,
                             kind="ExternalInput")
    wfc2_in = nc.dram_tensor("wfc2s", [HID, LATENT], F32,
                             kind="ExternalInput")
    bfc1_in = nc.dram_tensor("bfc1t", [N_GRAPHS, HID], F32,
                             kind="ExternalInput")
    bfc2_in = nc.dram_tensor("bfc2t", [N_GRAPHS, LATENT], F32,
                             kind="ExternalInput")
    out = nc.dram_tensor("out", [N_GRAPHS, LATENT], F32,
                         kind="ExternalOutput")

    # ---- internal DRAM ----
    tab2 = nc.dram_tensor("tab2", [N + 1, HID], BF16, addr_space="Shared")
    tab3 = nc.dram_tensor("tab3", [N + 1, HID], BF16, addr_space="Shared")
    cin2 = nc.dram_tensor("cin2", [OWN, HID], BF16)
    cin3 = nc.dram_tensor("cin3", [OWN, HID], BF16)
    hin = nc.dram_tensor("hin", [2, NODES_PER * HID], BF16)
    hfull = nc.dram_tensor("hfull", [N_GRAPHS, NODES_PER * HID], BF16, addr_space="Shared")
    arin = nc.dram_tensor("arin", [N_GRAPHS, LATENT], F32)
    arout = nc.dram_tensor("arout", [N_GRAPHS, LATENT], F32, addr_space="Shared")

    groups = [list(range(NC))]

    with tile.TileContext(nc) as tc:
        with tc.tile_pool(name="const", bufs=1) as cst, \
             tc.tile_pool(name="work", bufs=2) as wk, \
             tc.tile_pool(name="small", bufs=3) as sm, \
             tc.tile_pool(name="ps1", bufs=2, space="PSUM") as ps1, \
             tc.tile_pool(name="ps2", bufs=1, space="PSUM") as ps2, \
             tc.tile_pool(name="psfc", bufs=1, space="PSUM") as psfc, \
             tc.tile_pool(name="fcp", bufs=4) as fcp:

            # ---- constants to SBUF ----
            wbd_t = [cst.tile([P, P], BF16, tag=f"wbd{i}", name=f"wbd_t{i}")
                     for i in range(2)]
            for i in range(2):
                nc.sync.dma_start(out=wbd_t[i][:], in_=wbd_in[i])
            bst_t = [cst.tile([P, 1], F32, tag=f"bst{i}", name=f"bst_t{i}")
                     for i in range(3)]
            for i in range(3):
                nc.sync.dma_start(out=bst_t[i][:],
                                  in_=bst_in[i].rearrange("(p o) -> p o", o=1))
            idf_t = cst.tile([P, P], F32, tag="idf")
            nc.sync.dma_start(out=idf_t[:], in_=idf_in[:, :])
            idb_t = cst.tile([P, P], BF16, tag="idb")
            nc.sync.dma_start(out=idb_t[:], in_=idb_in[:, :])
            wfc2_t = cst.tile([HID, LATENT], F32, tag="wfc2")
            nc.sync.dma_start(out=wfc2_t[:], in_=wfc2_in[:, :])
            bfc1_t = cst.tile([N_GRAPHS, HID], F32, tag="bfc1")
            nc.sync.dma_start(out=bfc1_t[:], in_=bfc1_in[:, :])
            bfc2_t = cst.tile([N_GRAPHS, LATENT], F32, tag="bfc2")
            nc.sync.dma_start(out=bfc2_t[:], in_=bfc2_in[:, :])

            # zero row at index N for padding slots of tab2/tab3
            zrow = cst.tile([1, HID], BF16, tag="zrow")
            nc.gpsimd.memset(zrow[:], 0.0)
            nc.sync.dma_start(out=tab2[N:N + 1, :], in_=zrow[:])
            nc.sync.dma_start(out=tab3[N:N + 1, :], in_=zrow[:])

            # ---- conv layers ----
            def layer(li, table_ap, next_store):
                """li: 0,1,2. next_store(chunk_c, tnode_sbuf_tile)."""
                for c in range(NCHUNK):
                    it = sm.tile([P, PAD4], I32, tag="it")
                    nc.sync.dma_start(out=it[:], in_=idx_in[c])
                    wt = sm.tile([P, PAD4], BF16, tag="wt")
                    nc.sync.dma_start(out=wt[:], in_=w_in[c])
                    g = wk.tile([P, PAD4 * HID], BF16, tag="g")
                    for sl in range(PAD4):
                        nc.gpsimd.indirect_dma_start(
                            out=g[:, sl * HID:(sl + 1) * HID],
                            out_offset=None,
                            in_=table_ap,
                            in_offset=bass.IndirectOffsetOnAxis(
                                ap=it[:, sl:sl + 1], axis=0),
                        )
                    m = wk.tile([P, PAD4 * HID], BF16, tag="m")
                    nc.vector.tensor_tensor(
                        out=m[:].rearrange("p (c f) -> p c f", f=HID),
                        in0=g[:].rearrange("p (c f) -> p c f", f=HID),
                        in1=wt[:].to_broadcast(
                            [P, PAD4, HID]),
                        op=mybir.AluOpType.mult,
                    )
                    # tree-fold over s (outermost free factor), contiguous
                    BF = NB * HID
                    half = PAD // 2
                    m2 = wk.tile([P, half * BF], F32, tag="m2")
                    nc.vector.tensor_add(out=m2[:], in0=m[:, :half * BF],
                                         in1=m[:, half * BF:])
                    cur = half
                    while cur > 1:
                        h2 = cur // 2
                        nc.vector.tensor_add(out=m2[:, :h2 * BF],
                                             in0=m2[:, :h2 * BF],
                                             in1=m2[:, h2 * BF:cur * BF])
                        cur = h2
                    # ---- epilogue: [128 dst, (b f)] -> feature-major ----
                    pt = ps1.tile([P, P], F32, tag="pt")
                    nc.tensor.transpose(out=pt[:], in_=m2[:, :BF],
                                        identity=idf_t[:])
                    # ELU(v) with v = pt + b:  relu(v) + exp(min(v,0)) - 1
                    rl = sm.tile([P, P], BF16, tag="rl")
                    nc.scalar.activation(rl[:], pt[:],
                                         mybir.ActivationFunctionType.Relu,
                                         bias=bst_t[li][:])
                    mn = sm.tile([P, P], F32, tag="mn")
                    nc.vector.scalar_tensor_tensor(
                        out=mn[:], in0=pt[:], scalar=bst_t[li][:],
                        in1=rl[:], op0=mybir.AluOpType.add,
                        op1=mybir.AluOpType.subtract)
                    ex = sm.tile([P, P], F32, tag="ex")
                    nc.scalar.activation(ex[:], mn[:],
                                         mybir.ActivationFunctionType.Exp)
                    if li < 2:
                        h = sm.tile([P, P], BF16, tag="h")
                    else:
                        h = sm.tile([P, P], F32, tag="hf")
                    nc.vector.scalar_tensor_tensor(
                        out=h[:], in0=rl[:], scalar=-1.0, in1=ex[:],
                        op0=mybir.AluOpType.add, op1=mybir.AluOpType.add)
                    if li < 2:
                        # z = h @ W_{l+1} via block-diag W, still feat-major
                        pz = ps2.tile([P, P], F32, tag="pz")
                        nc.tensor.matmul(out=pz[:], lhsT=wbd_t[li][:],
                                         rhs=h[:], start=True, stop=True)
                        zs = sm.tile([P, P], F32, tag="zs")
                        nc.scalar.copy(zs[:], pz[:])
                        pn = ps1.tile([P, P], F32, tag="pn")
                        nc.tensor.transpose(out=pn[:], in_=zs[:],
                                            identity=idf_t[:])
                        tn = sm.tile([P, P], BF16, tag="tn")
                        nc.vector.tensor_copy(tn[:], pn[:])
                    else:
                        pn = ps1.tile([P, P], F32, tag="pn")
                        nc.tensor.transpose(out=pn[:], in_=h[:],
                                            identity=idf_t[:])
                        tn = sm.tile([P, P], BF16, tag="tn")
                        nc.vector.tensor_copy(tn[:], pn[:])
                    next_store(c, tn)

            # layer 1
            def store_l1(c, tn):
                nc.sync.dma_start(
                    out=cin2.ap()[c * CHD:(c + 1) * CHD, :].rearrange(
                        "(b p) f -> p b f", p=P),
                    in_=tn[:].rearrange("p (b f) -> p b f", f=HID))
            layer(0, tab1.ap(), store_l1)
            nc.gpsimd.collective_compute(
                "AllGather", mybir.AluOpType.bypass, replica_groups=groups,
                ins=[cin2.ap().opt()], outs=[tab2.ap()[:N, :].opt()])

            # layer 2
            def store_l2(c, tn):
                nc.sync.dma_start(
                    out=cin3.ap()[c * CHD:(c + 1) * CHD, :].rearrange(
                        "(b p) f -> p b f", p=P),
                    in_=tn[:].rearrange("p (b f) -> p b f", f=HID))
            layer(1, tab2.ap(), store_l2)
            nc.gpsimd.collective_compute(
                "AllGather", mybir.AluOpType.bypass, replica_groups=groups,
                ins=[cin3.ap().opt()], outs=[tab3.ap()[:N, :].opt()])

            # layer 3 -> H rows (2 graphs per core)
            def store_l3(c, tn):
                # tn[p, (b f)] -> h4[dloc = c*CHD + b*128 + p, f]
                nc.sync.dma_start(
                    out=hin.ap().rearrange("g (i f) -> (g i) f", f=HID)[
                        c * CHD:(c + 1) * CHD, :]
                    .rearrange("(b p) f -> p b f", p=P),
                    in_=tn[:].rearrange("p (b f) -> p b f", f=HID))
            layer(2, tab3.ap(), store_l3)
            nc.gpsimd.collective_compute(
                "AllGather", mybir.AluOpType.bypass, replica_groups=groups,
                ins=[hin.ap().opt()], outs=[hfull.ap().opt()])

            # ---- FC head ----
            # FC1: accumulate over 1024 dim-chunks of 128
            pfc = psfc.tile([N_GRAPHS, HID], F32, tag="pfc")
            NGRP = 8       # wfc1 chunks loaded per DMA
            NSLAB = 64     # H chunks per slab
            for cg in range(1024 // NGRP):
                wc = fcp.tile([P, NGRP * HID], BF16, tag="wc")
                nc.sync.dma_start(
                    out=wc[:],
                    in_=wfc1_in.ap()[:, cg * NGRP * HID:(cg + 1) * NGRP * HID])
                for j in range(NGRP):
                    ci = cg * NGRP + j
                    if ci % NSLAB == 0:
                        slab = fcp.tile([N_GRAPHS, NSLAB * P], BF16,
                                        tag="slab", name=f"slab{ci}")
                        nc.sync.dma_start(
                            out=slab[:],
                            in_=hfull.ap()[:, ci * P:(ci + NSLAB) * P])
                    pt16 = ps2.tile([P, N_GRAPHS], BF16, tag="pz",
                                    name=f"pt16_{ci}")
                    nc.tensor.transpose(
                        out=pt16[:],
                        in_=slab[:, (ci % NSLAB) * P:(ci % NSLAB + 1) * P],
                        identity=idb_t[:N_GRAPHS, :N_GRAPHS])
                    hc = fcp.tile([P, N_GRAPHS], BF16, tag="hc")
                    nc.vector.tensor_copy(hc[:], pt16[:])
                    nc.tensor.matmul(
                        out=pfc[:],
                        lhsT=hc[:],
                        rhs=wc[:, j * HID:(j + 1) * HID],
                        start=(ci == 0), stop=(ci == 1023))
            u = sm.tile([N_GRAPHS, HID], F32, tag="u")
            nc.vector.tensor_tensor(out=u[:], in0=pfc[:], in1=bfc1_t[:],
                                    op=mybir.AluOpType.add)
            rlu = sm.tile([N_GRAPHS, HID], F32, tag="rlu")
            nc.scalar.activation(rlu[:], u[:],
                                 mybir.ActivationFunctionType.Relu)
            mnu = sm.tile([N_GRAPHS, HID], F32, tag="mnu")
            nc.vector.scalar_tensor_tensor(
                out=mnu[:], in0=u[:], scalar=0.0, in1=rlu[:],
                op0=mybir.AluOpType.add, op1=mybir.AluOpType.subtract)
            exu = sm.tile([N_GRAPHS, HID], F32, tag="exu")
            nc.scalar.activation(exu[:], mnu[:],
                                 mybir.ActivationFunctionType.Exp)
            fcm = sm.tile([N_GRAPHS, HID], F32, tag="fcm")
            nc.vector.scalar_tensor_tensor(
                out=fcm[:], in0=rlu[:], scalar=-1.0, in1=exu[:],
                op0=mybir.AluOpType.add, op1=mybir.AluOpType.add)
            # transpose [16, 32] -> [32, 16]
            pT = psfc.tile([HID, N_GRAPHS], F32, tag="pT")
            nc.tensor.transpose(out=pT[:], in_=fcm[:],
                                identity=idf_t[:N_GRAPHS, :N_GRAPHS])
            fcmT = sm.tile([HID, N_GRAPHS], F32, tag="fcmT")
            nc.vector.tensor_copy(fcmT[:], pT[:])
            pP = psfc.tile([N_GRAPHS, LATENT], F32, tag="pP")
            nc.tensor.matmul(out=pP[:], lhsT=fcmT[:], rhs=wfc2_t[:],
                             start=True, stop=True)
            part = sm.tile([N_GRAPHS, LATENT], F32, tag="part")
            nc.vector.tensor_copy(part[:], pP[:])
            nc.sync.dma_start(out=arin.ap(), in_=part[:])
            nc.gpsimd.collective_compute(
                "AllReduce", mybir.AluOpType.add, replica_groups=groups,
                ins=[arin.ap().opt()], outs=[arout.ap().opt()])
            res = sm.tile([N_GRAPHS, LATENT], F32, tag="res")
            nc.sync.dma_start(out=res[:], in_=arout.ap())
            fin = sm.tile([N_GRAPHS, LATENT], F32, tag="fin")
            nc.vector.tensor_tensor(out=fin[:], in0=res[:], in1=bfc2_t[:],
                                    op=mybir.AluOpType.add)
            nc.sync.dma_start(out=out.ap(), in_=fin[:])

    nc.compile()
    return nc


def _host_prep(inputs):
    x = np.asarray(inputs["x"], np.float32)
    ei = np.asarray(inputs["edge_index"])
    w = np.asarray(inputs["edge_attr"], np.float32)
    W1 = np.asarray(inputs["W1"], np.float32)
    b1 = np.asarray(inputs["b1"], np.float32)
    W2 = np.asarray(inputs["W2"], np.float32)
    b2 = np.asarray(inputs["b2"], np.float32)
    W3 = np.asarray(inputs["W3"], np.float32)
    b3 = np.asarray(inputs["b3"], np.float32)
    Wfc1 = np.asarray(inputs["Wfc1"], np.float32)
    bfc1 = np.asarray(inputs["bfc1"], np.float32)
    Wfc2 = np.asarray(inputs["Wfc2"], np.float32)
    bfc2 = np.asarray(inputs["bfc2"], np.float32)

    src = ei[0].astype(np.int64)
    dst = ei[1].astype(np.int64)
    E = src.shape[0]

    order = np.argsort(dst, kind="stable")
    d_s = dst[order]
    s_s = src[order]
    w_s = w[order]
    deg = np.bincount(d_s, minlength=N)
    PAD = 8
    while PAD < int(deg.max()):
        PAD *= 2
    starts = np.zeros(N + 1, np.int64)
    np.cumsum(deg, out=starts[1:])
    pos = np.arange(E, dtype=np.int64) - starts[d_s]

    slot_idx = np.full((N, PAD), N, dtype=np.int32)
    slot_w = np.zeros((N, PAD), dtype=np.float32)
    slot_idx[d_s, pos] = s_s.astype(np.int32)
    slot_w[d_s, pos] = w_s

    # [core, chunk, b, p, s] -> [core, chunk, p, s, b]
    si = slot_idx.reshape(NC, NCHUNK, NB, P, PAD).transpose(0, 1, 3, 4, 2)
    idx_arr = np.ascontiguousarray(si.reshape(NC, NCHUNK, P, PAD * NB))
    sw = slot_w.reshape(NC, NCHUNK, NB, P, PAD).transpose(0, 1, 3, 4, 2)
    w_arr = np.ascontiguousarray(
        sw.reshape(NC, NCHUNK, P, PAD * NB)).astype(ml_dtypes.bfloat16)

    z1 = x @ W1
    tab1 = np.zeros((N + 1, HID), ml_dtypes.bfloat16)
    tab1[:N] = z1.astype(ml_dtypes.bfloat16)

    def blockdiag(W):
        out = np.zeros((P, P), np.float32)
        for t in range(NB):
            out[t * HID:(t + 1) * HID, t * HID:(t + 1) * HID] = W
        return out.astype(ml_dtypes.bfloat16)

    wbd = np.stack([blockdiag(W2), blockdiag(W3)])
    bst = np.stack([np.tile(b1, NB), np.tile(b2, NB),
                    np.tile(b3, NB)]).astype(np.float32)
    identf = np.eye(P, dtype=np.float32)

    in_maps = []
    for k in range(NC):
        wfc1s = np.ascontiguousarray(
            Wfc1[:, HID * k:HID * (k + 1)].reshape(1024, P, HID)
            .transpose(1, 0, 2).reshape(P, 1024 * HID)).astype(
                ml_dtypes.bfloat16)
        in_maps.append({
            "tab1": tab1,
            "idx": idx_arr[k],
            "warr": w_arr[k],
            "wbd": wbd,
            "bst": bst,
            "identf": identf,
            "identb": np.eye(P, dtype=ml_dtypes.bfloat16),
            "wfc1s": wfc1s,
            "wfc2s": np.ascontiguousarray(Wfc2[HID * k:HID * (k + 1), :]),
            "bfc1t": np.tile(bfc1[HID * k:HID * (k + 1)], (N_GRAPHS, 1)),
            "bfc2t": np.tile(bfc2, (N_GRAPHS, 1)),
        })
    return PAD, in_maps


def kernel(**inputs):
    PAD, in_maps = _host_prep(inputs)
    if PAD not in _prog_cache:
        _prog_cache[PAD] = _build_program(PAD)
    nc = _prog_cache[PAD]
    res = bass_utils.run_bass_kernel_spmd(nc, in_maps,
                                          core_ids=list(range(NC)))
    return np.asarray(res.results[0]["out"], np.float32)



# revision 22
# speedup vs baseline: 83.5793x; 83.5793x over previous
"""Trainium2 Bass kernel for nn_Encoder_88983132439258 (GNN message passing).

Strategy (8 NeuronCores, data-parallel over destination nodes = graphs):
  - Feature-major z-tables live in SBUF: T [128, 65536] bf16 where partition
    p holds feature p%32 of all 65536 nodes (4 replicas of z^T [32, N]).
  - Edge gather runs on GPSIMD ap_gather (d=2 "pair gather"): index
    src//2 (int16) fetches both nodes of a pair; the wrong pair element is
    killed by a host-built weight mask w2 (padding slots have w=0 too).
    One instruction gathers 4 dst-streams x 8192 slots.
  - Aggregation: one DVE tensor_tensor (m = g*w2) and one DVE tensor_reduce
    over the 2*PAD slot elements per dst -> agg [4s x 32f, 128 dst] f32.
  - Stream s of chunk c owns dsts [s*2048 + c*128, +128) so all stores are
    plain 2D APs (partition dim = (s,f) order matches DRAM row order).
  - ELU in feature-major (bias = per-partition), next-layer z via one
    block-diagonal (4x W) 128x128 matmul.
  - Tables are AllGathered ([128,2048] per core -> [1024,2048]) and
    reloaded into SBUF with 16 reshuffling DMAs.
  - FC head: layer-3 output is exchanged with AllToAll so core j owns
    H^T rows for its node-window of all 16 graphs; one dma_start_transpose
    forms all 128 lhsT chunks; 128 matmuls accumulate FC1 in PSUM;
    partials AllReduced; ELU+FC2 replicated on every core.
"""

import numpy as np
import ml_dtypes

import concourse.bacc as bacc
import concourse.mybir as mybir
import concourse.tile as tile
import concourse.bass as bass
from concourse import bass_utils

F32 = mybir.dt.float32
BF16 = mybir.dt.bfloat16
I16 = mybir.dt.int16

N = 65536
NODES_PER = 4096
N_GRAPHS = 16
FEAT_IN = 16
HID = 32
FC_HID = 256
LATENT = 64
NC = 8                 # cores
OWN = N // NC          # 8192 dsts per core
P = 128
NI = 8192              # gather indices per 16-partition group per chunk

_prog_cache = {}


def _build_program(PAD):
    DPS = NI // PAD        # dsts per stream per chunk (128 for PAD=64)
    SBLK = OWN // 4        # dsts per stream block (2048)
    NCHUNK = SBLK // DPS   # chunks per core (16)
    KFC = 16384            # FC1 contraction elems per core
    NCFC = KFC // P        # 128 matmul chunks

    nc = bacc.Bacc("TRN2", target_bir_lowering=False, debug=False,
                   num_devices=NC)

    # ---- I/O ----
    tabf_in = nc.dram_tensor("tabf", [HID, N], BF16, kind="ExternalInput")
    idx_in = nc.dram_tensor("idxp", [NCHUNK, P, NI // 16], I16,
                            kind="ExternalInput")
    w2_in = nc.dram_tensor("w2d", [NCHUNK, 4, 2 * NI], BF16,
                           kind="ExternalInput")
    wbd_in = nc.dram_tensor("wbd", [2, P, P], BF16, kind="ExternalInput")
    bst_in = nc.dram_tensor("bst", [3, P], F32, kind="ExternalInput")
    idf_in = nc.dram_tensor("identf", [N_GRAPHS, N_GRAPHS], F32,
                            kind="ExternalInput")
    wfc1_in = nc.dram_tensor("wfc1s", [KFC, FC_HID], BF16,
                             kind="ExternalInput")
    wfc2_in = nc.dram_tensor("wfc2s", [2 * P, LATENT], F32,
                             kind="ExternalInput")
    bfc1_in = nc.dram_tensor("bfc1t", [N_GRAPHS, FC_HID], F32,
                             kind="ExternalInput")
    bfc2_in = nc.dram_tensor("bfc2t", [N_GRAPHS, LATENT], F32,
                             kind="ExternalInput")
    out = nc.dram_tensor("out", [N_GRAPHS, LATENT], F32,
                         kind="ExternalOutput")

    # ---- internal DRAM ----
    # cinF[p, c*DPS+d]: partition-major local table chunk
    cinF = nc.dram_tensor("cinF", [P, NCHUNK * DPS], BF16)
    tabAG = nc.dram_tensor("tabAG", [NC * P, NCHUNK * DPS], BF16,
                           addr_space="Shared")
    # a2a block content: [(s f) p, q, d] per receiving core
    a2a_in = nc.dram_tensor("a2ain", [NC, P * 2 * DPS], BF16)
    a2a_out = nc.dram_tensor("a2aout", [NC, P * 2 * DPS], BF16)
    arin = nc.dram_tensor("arin", [N_GRAPHS, FC_HID], F32)
    arout = nc.dram_tensor("arout", [N_GRAPHS, FC_HID], F32,
                           addr_space="Shared")

    groups = [list(range(NC))]

    with tile.TileContext(nc) as tc:
        with tc.tile_pool(name="cst", bufs=1) as cst, \
             tc.tile_pool(name="big", bufs=1) as big, \
             tc.tile_pool(name="psA", bufs=1, space="PSUM") as psA, \
             tc.tile_pool(name="ps1", bufs=2, space="PSUM") as ps1, \
             tc.tile_pool(name="psfc", bufs=1, space="PSUM") as psfc:

            # ---- constants ----
            wbd_t = [cst.tile([P, P], BF16, tag=f"wbd{i}", name=f"wbd{i}")
                     for i in range(2)]
            for i in range(2):
                nc.sync.dma_start(out=wbd_t[i][:], in_=wbd_in[i])
            bst_t = [cst.tile([P, 1], F32, tag=f"bst{i}", name=f"bst{i}")
                     for i in range(3)]
            for i in range(3):
                nc.sync.dma_start(out=bst_t[i][:],
                                  in_=bst_in[i].rearrange("(p o) -> p o", o=1))
            idf_t = cst.tile([N_GRAPHS, N_GRAPHS], F32, tag="idf")
            nc.sync.dma_start(out=idf_t[:], in_=idf_in[:, :])
            wfc2_t = cst.tile([P, 2, LATENT], F32, tag="wfc2")
            nc.sync.dma_start(
                out=wfc2_t[:],
                in_=wfc2_in.ap().rearrange("(h p) o -> p h o", p=P))
            bfc1_t = cst.tile([N_GRAPHS, FC_HID], F32, tag="bfc1")
            nc.sync.dma_start(out=bfc1_t[:], in_=bfc1_in[:, :])
            bfc2_t = cst.tile([N_GRAPHS, LATENT], F32, tag="bfc2")
            nc.sync.dma_start(out=bfc2_t[:], in_=bfc2_in[:, :])

            # ---- SBUF table (4 replicas of z^T) ----
            T = big.tile([P, N], BF16, tag="T")
            for r in range(4):
                nc.sync.dma_start(out=T[HID * r:HID * (r + 1), :],
                                  in_=tabf_in[:, :])

            g = big.tile([P, 2 * NI], BF16, tag="g")
            w2 = big.tile([P, 2 * NI], BF16, tag="w2")
            it = big.tile([P, 2 * (NI // 16)], I16, tag="it")

            HB = NCHUNK // 2       # chunks per half-layer batch
            WID = HB * DPS         # 1024 wide-batch columns

            def conv_layer(li):
                for half in range(2):
                    pA = psA.tile([P, WID], F32, tag="pA")
                    for cr in range(HB):
                        c = half * HB + cr
                        if c % 2 == 0:
                            nc.sync.dma_start(
                                out=it[:].rearrange("p (q n) -> p q n", q=2),
                                in_=idx_in.ap()[c:c + 2].rearrange(
                                    "q p n -> p q n"))
                        itv = it[:, (c % 2) * (NI // 16):
                                 (c % 2 + 1) * (NI // 16)]
                        for s in range(4):
                            nc.sync.dma_start(
                                out=w2[HID * s:HID * (s + 1), :],
                                in_=w2_in.ap()[c][s:s + 1, :]
                                .to_broadcast((HID, 2 * NI)))
                        nc.gpsimd.ap_gather(
                            out_ap=g[:], in_ap=T[:], idxs_ap=itv,
                            channels=P, num_elems=N // 2, d=2, num_idxs=NI)
                        nc.vector.tensor_tensor(
                            out=g[:], in0=g[:], in1=w2[:],
                            op=mybir.AluOpType.mult)
                        nc.vector.tensor_reduce(
                            out=pA[:, cr * DPS:(cr + 1) * DPS],
                            in_=g[:].rearrange("p (d e) -> p d e",
                                               e=2 * PAD),
                            axis=mybir.AxisListType.X,
                            op=mybir.AluOpType.add)
                    # wide ELU(agg + b) on [128, WID]; scratch = w2 views
                    rlw = w2[:, 0:WID]
                    mnw = w2[:, WID:3 * WID].bitcast(F32)
                    exw = w2[:, 3 * WID:5 * WID].bitcast(F32)
                    hw = w2[:, 5 * WID:6 * WID]
                    tnw = w2[:, 6 * WID:7 * WID]
                    nc.scalar.activation(rlw, pA[:],
                                         mybir.ActivationFunctionType.Relu,
                                         bias=bst_t[li][:])
                    nc.vector.scalar_tensor_tensor(
                        out=mnw, in0=pA[:], scalar=bst_t[li][:],
                        in1=rlw, op0=mybir.AluOpType.add,
                        op1=mybir.AluOpType.subtract)
                    nc.scalar.activation(exw, mnw,
                                         mybir.ActivationFunctionType.Exp)
                    nc.vector.scalar_tensor_tensor(
                        out=hw, in0=rlw, scalar=-1.0, in1=exw,
                        op0=mybir.AluOpType.add, op1=mybir.AluOpType.add)
                    if li < 2:
                        for j2 in range(2):
                            pz = ps1.tile([P, WID // 2], F32, tag="pz")
                            nc.tensor.matmul(
                                out=pz[:], lhsT=wbd_t[li][:],
                                rhs=hw.rearrange(
                                    "p (j n) -> p j n", j=2)[:, j2],
                                start=True, stop=True)
                            nc.scalar.copy(
                                tnw.rearrange("p (j n) -> p j n",
                                              j=2)[:, j2], pz[:])
                        nc.sync.dma_start(
                            out=cinF.ap()[:, half * WID:(half + 1) * WID],
                            in_=tnw)
                    else:
                        for q in range(2):
                            nc.sync.dma_start(
                                out=a2a_in.ap().rearrange(
                                    "i (p q d) -> q i p d",
                                    p=P, q=2)[q][4 * half:4 * (half + 1)]
                                .rearrange("i p d -> p i d"),
                                in_=hw.rearrange(
                                    "p (i q d) -> p q i d",
                                    q=2, d=DPS)[:, q])

                if li < 2:
                    nc.gpsimd.collective_compute(
                        "AllGather", mybir.AluOpType.bypass,
                        replica_groups=groups,
                        ins=[cinF.ap().opt()], outs=[tabAG.ap().opt()])
                    for r in range(4):
                        for s in range(4):
                            nc.sync.dma_start(
                                out=T[HID * r:HID * (r + 1), :].rearrange(
                                    "f (k s cd) -> f k s cd",
                                    k=NC, s=4)[:, :, s],
                                in_=tabAG.ap().rearrange(
                                    "(k s f) cd -> f k s cd",
                                    s=4, f=HID)[:, :, s])

            conv_layer(0)
            conv_layer(1)
            conv_layer(2)

            nc.gpsimd.collective_compute(
                "AllToAll", mybir.AluOpType.bypass, replica_groups=groups,
                ins=[a2a_in.ap().opt()], outs=[a2a_out.ap().opt()])

            # ---- FC head ----
            wfc_full = big.tile([P, N], BF16, tag="T")  # reuse T buffer
            wfc = wfc_full[:, :NCFC * FC_HID].rearrange(
                "p (c o) -> p c o", o=FC_HID)
            nc.sync.dma_start(
                out=wfc,
                in_=wfc1_in.ap().rearrange("(c p) o -> p c o", p=P))
            lhsT = cst.tile([P, NCFC, N_GRAPHS], BF16, tag="lhsT")
            nc.sync.dma_start_transpose(
                out=lhsT[:],
                in_=a2a_out.ap().rearrange("j (gl x) -> (j gl) x", gl=2))
            pfc = psfc.tile([N_GRAPHS, FC_HID], F32, tag="pfc")
            for c in range(NCFC):
                nc.tensor.matmul(out=pfc[:], lhsT=lhsT[:, c, :],
                                 rhs=wfc[:, c, :],
                                 start=(c == 0), stop=(c == NCFC - 1))
            # FC scratch: f32 views of the (dead) g buffer
            part = g[:N_GRAPHS, 0:512].bitcast(F32)
            u2 = g[:N_GRAPHS, 512:1024].bitcast(F32)
            rl2 = g[:N_GRAPHS, 1024:1536].bitcast(F32)
            mn2 = g[:N_GRAPHS, 1536:2048].bitcast(F32)
            ex2 = g[:N_GRAPHS, 2048:2560].bitcast(F32)
            fin = g[:N_GRAPHS, 2624:2752].bitcast(F32)
            nc.vector.tensor_copy(part, pfc[:])
            nc.sync.dma_start(out=arin.ap(), in_=part)
            nc.gpsimd.collective_compute(
                "AllReduce", mybir.AluOpType.add, replica_groups=groups,
                ins=[arin.ap().opt()], outs=[arout.ap().opt()])
            nc.sync.dma_start(out=part, in_=arout.ap())
            nc.vector.tensor_tensor(out=u2, in0=part, in1=bfc1_t[:],
                                    op=mybir.AluOpType.add)
            nc.scalar.activation(rl2, u2,
                                 mybir.ActivationFunctionType.Relu)
            nc.vector.scalar_tensor_tensor(
                out=mn2, in0=u2, scalar=0.0, in1=rl2,
                op0=mybir.AluOpType.add, op1=mybir.AluOpType.subtract)
            nc.scalar.activation(ex2, mn2,
                                 mybir.ActivationFunctionType.Exp)
            fcm = u2
            nc.vector.scalar_tensor_tensor(
                out=fcm, in0=rl2, scalar=-1.0, in1=ex2,
                op0=mybir.AluOpType.add, op1=mybir.AluOpType.add)
            pP = psfc.tile([N_GRAPHS, LATENT], F32, tag="pP")
            for hh in range(2):
                tp = ps1.tile([P, N_GRAPHS], F32, tag="tp",
                              name=f"tp{hh}")
                nc.tensor.transpose(out=tp[:],
                                    in_=fcm[:, hh * P:(hh + 1) * P],
                                    identity=idf_t[:])
                tcp = g[:, 2560 + hh * 32:2560 + (hh + 1) * 32].bitcast(F32)
                nc.vector.tensor_copy(tcp, tp[:])
                nc.tensor.matmul(out=pP[:], lhsT=tcp,
                                 rhs=wfc2_t[:, hh, :],
                                 start=(hh == 0), stop=(hh == 1))
            nc.vector.tensor_tensor(out=fin, in0=pP[:], in1=bfc2_t[:],
                                    op=mybir.AluOpType.add)
            nc.sync.dma_start(out=out.ap(), in_=fin)

    nc.compile()
    return nc


def _host_prep(inputs):
    x = np.asarray(inputs["x"], np.float32)
    ei = np.asarray(inputs["edge_index"])
    w = np.asarray(inputs["edge_attr"], np.float32)
    W1 = np.asarray(inputs["W1"], np.float32)
    b1 = np.asarray(inputs["b1"], np.float32)
    W2 = np.asarray(inputs["W2"], np.float32)
    b2 = np.asarray(inputs["b2"], np.float32)
    W3 = np.asarray(inputs["W3"], np.float32)
    b3 = np.asarray(inputs["b3"], np.float32)
    Wfc1 = np.asarray(inputs["Wfc1"], np.float32)
    bfc1 = np.asarray(inputs["bfc1"], np.float32)
    Wfc2 = np.asarray(inputs["Wfc2"], np.float32)
    bfc2 = np.asarray(inputs["bfc2"], np.float32)

    src = ei[0].astype(np.int64)
    dst = ei[1].astype(np.int64)
    E = src.shape[0]

    order = np.argsort(dst, kind="stable")
    d_s = dst[order]
    s_s = src[order]
    w_s = w[order]
    deg = np.bincount(d_s, minlength=N)
    PAD = 8
    while PAD < int(deg.max()):
        PAD *= 2
    starts = np.zeros(N + 1, np.int64)
    np.cumsum(deg, out=starts[1:])
    pos = np.arange(E, dtype=np.int64) - starts[d_s]

    slot_idx = np.zeros((N, PAD), dtype=np.int32)
    slot_w = np.zeros((N, PAD), dtype=np.float32)
    slot_idx[d_s, pos] = s_s.astype(np.int32)
    slot_w[d_s, pos] = w_s

    assert PAD == 64, f"kernel geometry assumes PAD=64, got {PAD}"
    DPS = NI // PAD
    NCHUNK = (OWN // 4) // DPS

    pair = (slot_idx >> 1).astype(np.int16)        # [N, PAD]
    parity = (slot_idx & 1).astype(np.int8)

    # dst of (core, stream, chunk, d) = core*OWN + s*2048 + c*DPS + d
    # slot i = d*PAD + sl
    pr_c = pair.reshape(NC, 4, NCHUNK, DPS * PAD).transpose(0, 2, 1, 3)
    pa_c = parity.reshape(NC, 4, NCHUNK, DPS * PAD).transpose(0, 2, 1, 3)
    w_c = slot_w.reshape(NC, 4, NCHUNK, DPS * PAD).transpose(0, 2, 1, 3)

    # wrapped indices: [NC, NCHUNK, 128, NI//16]; i = col*16 + part
    iw = pr_c.reshape(NC, NCHUNK, 4, NI // 16, 16)
    iw = np.ascontiguousarray(iw.transpose(0, 1, 2, 4, 3))
    idx_wrapped = np.repeat(iw, 2, axis=2).reshape(NC, NCHUNK, P, NI // 16)

    # w2: [NC, NCHUNK, 4, 2*NI]  col = i*2 + j, kill wrong pair element
    w2d = np.zeros((NC, NCHUNK, 4, NI, 2), np.float32)
    np.put_along_axis(w2d, np.ascontiguousarray(pa_c)[..., None]
                      .astype(np.int64), np.ascontiguousarray(w_c)[..., None],
                      axis=4)
    w2d = w2d.reshape(NC, NCHUNK, 4, 2 * NI).astype(ml_dtypes.bfloat16)

    tabf = np.ascontiguousarray((x @ W1).T).astype(ml_dtypes.bfloat16)

    def blockdiag(W):
        out_ = np.zeros((P, P), np.float32)
        for t in range(4):
            out_[t * HID:(t + 1) * HID, t * HID:(t + 1) * HID] = W
        return out_.astype(ml_dtypes.bfloat16)

    wbd = np.stack([blockdiag(W2), blockdiag(W3)])
    bst = np.stack([np.tile(b1, 4), np.tile(b2, 4),
                    np.tile(b3, 4)]).astype(np.float32)
    identf = np.eye(N_GRAPHS, dtype=np.float32)

    # FC1 per-core slice: k' = sp*8192 + f*256 + q*128 + d maps to global
    # node ng = sp*2048 + (2j+q)*128 + d of every graph, feature f.
    # Wfc1 [131072, 256] -> [sp 2, m 32, d 128, f 32, o]
    wfc1_r = Wfc1.reshape(2, 16, DPS, HID, FC_HID)   # [sp, m, d, f, o]
    in_maps = []
    for k in range(NC):
        # receiver k gets senders' chunks c = 2k, 2k+1 -> m = 2k + q
        wj = wfc1_r[:, 2 * k:2 * k + 2]              # [sp, q 2, d, f, o]
        wj = np.ascontiguousarray(wj.transpose(0, 3, 1, 2, 4)).reshape(
            KFC_HOST, FC_HID).astype(ml_dtypes.bfloat16)
        in_maps.append({
            "tabf": tabf,
            "idxp": idx_wrapped[k],
            "w2d": w2d[k],
            "wbd": wbd,
            "bst": bst,
            "identf": identf,
            "wfc1s": wj,
            "wfc2s": np.ascontiguousarray(Wfc2.astype(np.float32)),
            "bfc1t": np.tile(bfc1, (N_GRAPHS, 1)),
            "bfc2t": np.tile(bfc2, (N_GRAPHS, 1)),
        })
    return PAD, in_maps


KFC_HOST = 16384


def kernel(**inputs):
    PAD, in_maps = _host_prep(inputs)
    if PAD not in _prog_cache:
        _prog_cache[PAD] = _build_program(PAD)
    nc = _prog_cache[PAD]
    res = bass_utils.run_bass_kernel_spmd(nc, in_maps,
                                          core_ids=list(range(NC)))
    return np.asarray(res.results[0]["out"], np.float32)
